# revision 18
# baseline (speedup 1.0000x reference)
"""Trainium2 Bass kernel for nn_ExperimentalLayer9 (dense transformer layer).

Layer: x + gelu(attn(x) ) @ Wf with
  Q = split_heads(x), K = split_heads(x@Wk+bk), V = split_heads(x@Wv+bv)
  causal softmax (no 1/sqrt(d) scale), exact-erf gelu, residual add.

Sharding over 8 NeuronCores: 2 batch groups x 4-way head/tensor parallel.
Core c handles batch b=c//4 and heads [4r, 4r+4) with r=c%4.  Each core
computes K^T/V projections for its head slice, causal flash-style
attention in transposed-score layout, gelu, and a partial FF over its
1024-row slice of Wf.  A 4-rank ReduceScatter (bf16) sums the FF
partials within each batch group; each core adds the residual x rows for
its rank's 512-row shard and returns that shard.  The host reassembles
the [2, 2048, 1024] output.

All matmuls run in bf16 (fp32 PSUM accumulation); softmax/normalization
in fp32.  exp is computed without max-subtraction (scores are bounded:
std ~5, so exp stays well inside fp32/bf16 range) which avoids any
partition-axis max reduction.  The exp-sum l(q) is obtained for free by
appending a ones-column to V in the attention@V matmul; 1/l is then a
per-partition scalar multiply fused on the vector engine.
"""

import numpy as np
import ml_dtypes

import concourse.bass as bass
import concourse.mybir as mybir
import concourse.tile as tile
from concourse import bacc
from concourse import bass_utils

# Problem shapes (hardcoded per contest contract).
B, S, D, H, DHID = 2, 2048, 1024, 16, 4096
NCORES = 8
GROUP = 4              # cores per batch group
HPC = 4                # heads per core
DK = 64                # q/k head dim
DV = 256               # v head dim
DKS = HPC * DK         # 256  k-slice per core
DVS = HPC * DV         # 1024 v/hidden slice per core
ROWS = S // GROUP      # 512  output rows per core after ReduceScatter
NM = D // 128          # 8    contraction chunks over d_model
VSTRIDE = DV + 1       # 257  V columns per head incl. ones column

BF16 = mybir.dt.bfloat16
F32 = mybir.dt.float32
AF = mybir.ActivationFunctionType

bf16 = ml_dtypes.bfloat16

_compiled = None


def build_program():
    nc = bacc.Bacc(
        "TRN2",
        target_bir_lowering=False,
        debug=False,
        enable_asserts=True,
        num_devices=NCORES,
    )

    # Per-core inputs (values differ per core; program is SPMD-identical).
    xT = nc.dram_tensor("xT", [D, S], BF16, kind="ExternalInput").ap()
    qT = nc.dram_tensor("qT", [DKS, S], BF16, kind="ExternalInput").ap()
    xres = nc.dram_tensor("xres", [ROWS, D], F32, kind="ExternalInput").ap()
    wk = nc.dram_tensor("wk", [D, DKS], BF16, kind="ExternalInput").ap()
    wv = nc.dram_tensor("wv", [D, DVS], BF16, kind="ExternalInput").ap()
    wf = nc.dram_tensor("wf", [DVS, D], BF16, kind="ExternalInput").ap()
    bkb = nc.dram_tensor("bkb", [1, DKS], BF16, kind="ExternalInput").ap()
    bvb = nc.dram_tensor("bvb", [1, DVS], BF16, kind="ExternalInput").ap()
    maskt = nc.dram_tensor("maskt", [128, 128], BF16, kind="ExternalInput").ap()
    ident = nc.dram_tensor("ident", [128, 128], BF16, kind="ExternalInput").ap()
    onesr = nc.dram_tensor("onesr", [1, 512], BF16, kind="ExternalInput").ap()
    out = nc.dram_tensor("out", [ROWS, D], F32, kind="ExternalOutput").ap()

    with tile.TileContext(nc) as tc:
        _body(nc, tc, xT, qT, xres, wk, wv, wf, bkb, bvb, maskt, ident, onesr, out)

    nc.compile()
    return nc


def _body(nc, tc, xT, qT, xres, wk, wv, wf, bkb, bvb, maskt, ident, onesr, out):
    NST = S // 128     # 16 s tiles of 128
    NQT2 = S // 1024   # 2  q tiles of 1024

    with (
        tc.tile_pool(name="const", bufs=1) as constp,
        tc.tile_pool(name="kv", bufs=1) as kvp,
        tc.tile_pool(name="got", bufs=1) as gotp,
        tc.tile_pool(name="res", bufs=1) as resp,
        tc.tile_pool(name="rfp", bufs=2) as rfp,
        tc.tile_pool(name="small", bufs=8) as smallp,
        tc.tile_pool(name="dram", bufs=1, space="DRAM") as dramp,
    ):
        # ---- constants (ACT queue) ------------------------------------
        ones_sb = constp.tile([1, 512], BF16)
        nc.scalar.dma_start(ones_sb[:], onesr[:])
        mask_sb = constp.tile([128, 128], BF16)
        nc.scalar.dma_start(mask_sb[:], maskt[:])
        bk_sb = constp.tile([1, DKS], BF16)
        nc.scalar.dma_start(bk_sb[:], bkb[:])
        bv_sb = constp.tile([1, DVS], BF16)
        nc.scalar.dma_start(bv_sb[:], bvb[:])

        # Warm up the collectives path (ncfw/channel setup) so the first
        # real ReduceScatter doesn't pay ~25us of first-call overhead.
        warm_in = dramp.tile([4, 16], BF16, tag="warm_in")
        warm_out = dramp.tile([1, 16], BF16, tag="warm_out")
        nc.scalar.dma_start(
            warm_in[:].rearrange("a b -> (a b)")[None, :], ones_sb[0:1, 0:64]
        )
        nc.gpsimd.collective_compute(
            "ReduceScatter",
            mybir.AluOpType.add,
            replica_groups=[[0, 1, 2, 3], [4, 5, 6, 7]],
            ins=[warm_in.opt()],
            outs=[warm_out.opt()],
        )

        # [1024, n] DRAM -> [128, 8*n] SBUF, per-chunk DMAs on the Sync
        # queue (all complete before the first xbar transpose issues)
        def load_chunked(pool, src, n):
            t = pool.tile([128, NM * n], src.dtype)
            for m in range(NM):
                nc.sync.dma_start(
                    t[:, m * n : (m + 1) * n],
                    src[m * 128 : (m + 1) * 128, :],
                )
            return t

        qT_sb = kvp.tile([128, 2 * S], BF16)
        for m in range(2):
            nc.sync.dma_start(
                qT_sb[:, m * S : (m + 1) * S], qT[m * 128 : (m + 1) * 128, :]
            )
        kt_sb = kvp.tile([128, 2 * S], BF16)   # K^T rows dk%128, chunk dk//128
        v_sb = kvp.tile([128, NST * HPC * VSTRIDE], BF16)
        got_sb = gotp.tile([128, NM * S], BF16)  # gelu(o)^T, hc-major x q
        # residual x rows: no deps, load early (ACT queue)
        xrs = []
        for g in range(4):
            xr = resp.tile([128, D], F32, tag=f"xr{g}")
            nc.scalar.dma_start(xr[:], xres[g * 128 : (g + 1) * 128, :])
            xrs.append(xr)

        # ---- projections ---------------------------------------------
        with (
            tc.tile_pool(name="projw", bufs=1) as pwp,
            tc.tile_pool(name="xt", bufs=1) as xtp,
            tc.tile_pool(name="psProj", bufs=4, space="PSUM") as psP,
        ):
            wk_sb = load_chunked(pwp, wk, DKS)
            xT_sb = load_chunked(xtp, xT, S)
            wv_sb = load_chunked(pwp, wv, DVS)

            # K^T[dk, s]: lhsT = Wk chunk [128m, 128dk], rhs = xT chunk [128m, 512s]
            for dkt in range(2):
                for st in range(4):
                    ps = psP.tile([128, 512], F32, tag="proj")
                    nc.tensor.matmul(
                        ps[:],
                        bk_sb[:, dkt * 128 : (dkt + 1) * 128],
                        ones_sb[:, 0:512],
                        start=True,
                        stop=False,
                    )
                    for m in range(NM):
                        nc.tensor.matmul(
                            ps[:],
                            wk_sb[:, m * DKS + dkt * 128 : m * DKS + dkt * 128 + 128],
                            xT_sb[:, m * S + st * 512 : m * S + st * 512 + 512],
                            start=False,
                            stop=(m == NM - 1),
                        )
                    nc.scalar.copy(
                        kt_sb[:, dkt * S + st * 512 : dkt * S + st * 512 + 512], ps[:]
                    )

            # V[s, dv] with a ones column per head (col 256 of each strip)
            nc.vector.memset(
                v_sb[:].rearrange("p (t h c) -> p t h c", t=NST, h=HPC)[:, :, :, DV],
                1.0,
            )
            for st in range(NST):
                for dvh in range(2):  # dv halves of 512 = heads (2*dvh, 2*dvh+1)
                    ps = psP.tile([128, 512], F32, tag="proj")
                    nc.tensor.matmul(
                        ps[:],
                        ones_sb[:, 0:128],
                        bv_sb[:, dvh * 512 : dvh * 512 + 512],
                        start=True,
                        stop=False,
                    )
                    for m in range(NM):
                        nc.tensor.matmul(
                            ps[:],
                            xT_sb[:, m * S + st * 128 : m * S + st * 128 + 128],
                            wv_sb[:, m * DVS + dvh * 512 : m * DVS + dvh * 512 + 512],
                            start=False,
                            stop=(m == NM - 1),
                        )
                    base = st * HPC * VSTRIDE
                    for hh in range(2):
                        h = 2 * dvh + hh
                        nc.scalar.copy(
                            v_sb[:, base + h * VSTRIDE : base + h * VSTRIDE + DV],
                            ps[:, hh * 256 : hh * 256 + 256],
                        )

        # ---- attention (head pairs, row-tiled scores) ----------------
        # scores^T[k, q]: contraction is dk=64, so heads 2p (PE rows 0-63)
        # and 2p+1 (rows 64-127) run concurrently via tile_position row
        # tiling.  AV groups run in default 128x128 mode afterwards;
        # exp without max-subtraction; o tiles transposed by xbar DMA.
        with (
            tc.tile_pool(name="expp", bufs=1) as expp,
            tc.tile_pool(name="otile", bufs=4) as otp,
            tc.tile_pool(name="psSt", bufs=3, space="PSUM") as psS,
            tc.tile_pool(name="psAv", bufs=2, space="PSUM") as psV,
        ):
            NQT2 = S // 1024
            for pair in range(2):
                co = pair * S           # both heads of the pair share chunk co

                def st_tile(j, kt, hl, exps):
                    po = 64 * (hl % 2)
                    t = kt - 8 * j   # >=0 on diagonal k-tiles
                    toff = max(t, 0) * 128
                    q0 = j * 1024 + toff
                    ps = psS.tile([128, 1024], F32, tag="st")
                    lo_w = max(0, 512 - toff)
                    if lo_w:
                        nc.tensor.matmul(
                            ps[:, toff : toff + lo_w],
                            kt_sb[po : po + 64, co + kt * 128 : co + kt * 128 + 128],
                            qT_sb[po : po + 64, co + q0 : co + q0 + lo_w],
                            start=True,
                            stop=True,
                            tile_position=(po, 0),
                        )
                    nc.tensor.matmul(
                        ps[:, max(toff, 512) : 1024],
                        kt_sb[po : po + 64, co + kt * 128 : co + kt * 128 + 128],
                        qT_sb[po : po + 64, co + j * 1024 + max(toff, 512) : co + (j + 1) * 1024],
                        start=True,
                        stop=True,
                        tile_position=(po, 0),
                    )
                    nc.scalar.activation(
                        exps[:, kt * 1024 + toff : (kt + 1) * 1024],
                        ps[:, toff:1024],
                        AF.Exp,
                    )
                    if t >= 0:  # mask the diagonal 128x128 block
                        blk = exps[:, kt * 1024 + toff : kt * 1024 + toff + 128]
                        nc.vector.tensor_mul(blk, blk, mask_sb[:])

                def av_tile(j, sq, hl, exps):
                    i = 8 * j + sq
                    pso = psV.tile([128, VSTRIDE], F32, tag="av")
                    for kt in range(i + 1):
                        vb = kt * HPC * VSTRIDE + hl * VSTRIDE
                        nc.tensor.matmul(
                            pso[:],
                            exps[:, kt * 1024 + sq * 128 : kt * 1024 + sq * 128 + 128],
                            v_sb[:, vb : vb + VSTRIDE],
                            start=(kt == 0),
                            stop=(kt == i),
                        )
                    recip = smallp.tile([128, 1], F32, tag="recip")
                    nc.vector.reciprocal(recip[:], pso[:, DV : DV + 1])
                    ot = otp.tile([128, DV], BF16, tag="ot")
                    nc.vector.tensor_scalar_mul(ot[:], pso[:, 0:DV], recip[:])
                    for half in range(2):
                        hc = 2 * hl + half
                        nc.sync.dma_start_transpose(
                            got_sb[:, hc * S + i * 128 : hc * S + i * 128 + 128],
                            ot[:, half * 128 : half * 128 + 128],
                        )

                for j in range(NQT2):   # 1024-wide q tiles
                    hA, hB = 2 * pair, 2 * pair + 1
                    exps_a = expp.tile([128, 16 * 1024], BF16, tag="expSA")
                    exps_b = expp.tile([128, 16 * 1024], BF16, tag="expSB")
                    # row-tiled score phase: both heads stream concurrently
                    for kt in range(8 * j + 8):
                        st_tile(j, kt, hA, exps_a)
                        st_tile(j, kt, hB, exps_b)
                    # default-mode AV phase
                    for sq in range(8):
                        av_tile(j, sq, hA, exps_a)
                        av_tile(j, sq, hB, exps_b)

        # ---- gelu (exact erf) in place on transposed layout ----------
        for hc in range(NM):
            nc.scalar.activation(
                got_sb[:, hc * S : (hc + 1) * S],
                got_sb[:, hc * S : (hc + 1) * S],
                AF.Gelu,
            )

        # ---- FF partial + chunked ReduceScatter + gpsimd residual ----
        with (
            tc.tile_pool(name="ffw", bufs=1) as ffwp,
            tc.tile_pool(name="ffout", bufs=4) as ffoutp,
            tc.tile_pool(name="psFf", bufs=3, space="PSUM") as psF,
        ):
            wf_sb = load_chunked(ffwp, wf, D)
            for g in range(2):
                partial_d = dramp.tile([1024, D], BF16, tag=f"part{g}")
                for cc in range(8):
                    c = 8 * g + cc
                    ps0 = psF.tile([128, 512], F32, tag="ff0")
                    ps1 = psF.tile([128, 512], F32, tag="ff1")
                    for hc in range(NM):
                        lhsT = got_sb[:, hc * S + c * 128 : hc * S + c * 128 + 128]
                        nc.tensor.matmul(
                            ps0[:], lhsT, wf_sb[:, hc * D : hc * D + 512],
                            start=(hc == 0), stop=(hc == NM - 1),
                        )
                        nc.tensor.matmul(
                            ps1[:], lhsT, wf_sb[:, hc * D + 512 : hc * D + 1024],
                            start=(hc == 0), stop=(hc == NM - 1),
                        )
                    fo = ffoutp.tile([128, D], BF16, tag="ffout")
                    nc.vector.tensor_copy(fo[:, 0:512], ps0[:])
                    nc.vector.tensor_copy(fo[:, 512:1024], ps1[:])
                    nc.scalar.dma_start(partial_d[cc * 128 : (cc + 1) * 128, :], fo[:])
                rs_d = dramp.tile([256, D], BF16, tag=f"rs{g}")
                nc.gpsimd.collective_compute(
                    "ReduceScatter",
                    mybir.AluOpType.add,
                    replica_groups=[[0, 1, 2, 3], [4, 5, 6, 7]],
                    ins=[partial_d.opt()],
                    outs=[rs_d.opt()],
                )
                # residual: RS-gated cast-DMA on the GpSimd queue, add on
                # DVE, store on ACT
                for part in range(2):
                    tix = 2 * g + part
                    rf = rfp.tile([128, D], F32, tag="rf")
                    nc.gpsimd.dma_start(rf[:], rs_d[part * 128 : (part + 1) * 128, :])
                    nc.vector.tensor_add(xrs[tix][:], xrs[tix][:], rf[:])
                    nc.scalar.dma_start(out[tix * 128 : (tix + 1) * 128, :], xrs[tix][:])


def make_in_maps(x, Wk, bk, Wv, bv, Wf, bf):
    """Host-side sharding: returns the per-core input dict list."""
    x = np.asarray(x, np.float32)
    Wk = np.asarray(Wk, np.float32)
    Wv = np.asarray(Wv, np.float32)
    Wf = np.asarray(Wf, np.float32)
    bk = np.asarray(bk, np.float32)
    bv = np.asarray(bv, np.float32)
    bf = np.asarray(bf, np.float32)
    mask = np.tril(np.ones((128, 128), np.float32)).T  # mask[k,q]=1 iff k<=q
    in_maps = []
    for c in range(NCORES):
        b, r = c // GROUP, c % GROUP
        xb = x[b]                                    # [S, D]
        xT = np.ascontiguousarray(xb.T).astype(bf16)
        qTs = xT[DKS * r : DKS * (r + 1)]            # heads 4r..4r+3 rows
        # chunked RS: core (b,r) tile t=2g+p holds x rows 1024g+256r+128p
        xres = np.concatenate(
            [
                xb[1024 * g + 256 * r + 128 * p : 1024 * g + 256 * r + 128 * p + 128]
                for g in range(2)
                for p in range(2)
            ]
        ) + bf[None, :].astype(np.float32)
        in_maps.append({
            "xT": xT,
            "qT": np.ascontiguousarray(qTs),
            "xres": np.ascontiguousarray(xres),
            "wk": np.ascontiguousarray(Wk[:, DKS * r : DKS * (r + 1)]).astype(bf16),
            "wv": np.ascontiguousarray(Wv[:, DVS * r : DVS * (r + 1)]).astype(bf16),
            "wf": np.ascontiguousarray(Wf[DVS * r : DVS * (r + 1), :]).astype(bf16),
            "bkb": bk[None, DKS * r : DKS * (r + 1)].astype(bf16),
            "bvb": bv[None, DVS * r : DVS * (r + 1)].astype(bf16),
            "maskt": mask.astype(bf16),
            "ident": np.eye(128, dtype=np.float32).astype(bf16),
            "onesr": np.ones((1, 512), bf16),
        })
    return in_maps


def assemble(results):
    """[8 x [512,1024]] core outputs -> [2,2048,1024]."""
    out = np.empty((B, S, D), np.float32)
    for c in range(NCORES):
        b, r = c // GROUP, c % GROUP
        for g in range(2):
            for p in range(2):
                t = 2 * g + p
                out[b, 1024 * g + 256 * r + 128 * p : 1024 * g + 256 * r + 128 * p + 128, :] = (
                    results[c]["out"][128 * t : 128 * (t + 1)]
                )
    return out


def kernel(x, Wk, bk, Wv, bv, Wf, bf, _trace=False, _trace_cores=None):
    global _compiled
    if _compiled is None:
        _compiled = build_program()
    nc = _compiled
    in_maps = make_in_maps(x, Wk, bk, Wv, bv, Wf, bf)
    res = bass_utils.run_bass_kernel_spmd(
        nc,
        in_maps,
        core_ids=list(range(NCORES)),
        trace=_trace,
        trace_cores=_trace_cores,
    )
    out = assemble(res.results)
    kernel.last_result = res
    return out


# revision 19
# speedup vs baseline: 1.0410x; 1.0410x over previous
"""Trainium2 Bass kernel for nn_ExperimentalLayer9 (dense transformer layer).

Layer: x + gelu(attn(x) ) @ Wf with
  Q = split_heads(x), K = split_heads(x@Wk+bk), V = split_heads(x@Wv+bv)
  causal softmax (no 1/sqrt(d) scale), exact-erf gelu, residual add.

Sharding over 8 NeuronCores: 2 batch groups x 4-way head/tensor parallel.
Core c handles batch b=c//4 and heads [4r, 4r+4) with r=c%4.  Each core
computes K^T/V projections for its head slice, causal flash-style
attention in transposed-score layout, gelu, and a partial FF over its
1024-row slice of Wf.  A 4-rank ReduceScatter (bf16) sums the FF
partials within each batch group; each core adds the residual x rows for
its rank's 512-row shard and returns that shard.  The host reassembles
the [2, 2048, 1024] output.

All matmuls run in bf16 (fp32 PSUM accumulation); softmax/normalization
in fp32.  exp is computed without max-subtraction (scores are bounded:
std ~5, so exp stays well inside fp32/bf16 range) which avoids any
partition-axis max reduction.  The exp-sum l(q) is obtained for free by
appending a ones-column to V in the attention@V matmul; 1/l is then a
per-partition scalar multiply fused on the vector engine.
"""

import numpy as np
import ml_dtypes

import concourse.bass as bass
import concourse.mybir as mybir
import concourse.tile as tile
from concourse import bacc
from concourse import bass_utils

# Problem shapes (hardcoded per contest contract).
B, S, D, H, DHID = 2, 2048, 1024, 16, 4096
NCORES = 8
GROUP = 4              # cores per batch group
HPC = 4                # heads per core
DK = 64                # q/k head dim
DV = 256               # v head dim
DKS = HPC * DK         # 256  k-slice per core
DVS = HPC * DV         # 1024 v/hidden slice per core
ROWS = S // GROUP      # 512  output rows per core after ReduceScatter
NM = D // 128          # 8    contraction chunks over d_model
VSTRIDE = DV + 1       # 257  V columns per head incl. ones column

BF16 = mybir.dt.bfloat16
F32 = mybir.dt.float32
AF = mybir.ActivationFunctionType

bf16 = ml_dtypes.bfloat16

_compiled = None


def build_program():
    nc = bacc.Bacc(
        "TRN2",
        target_bir_lowering=False,
        debug=False,
        enable_asserts=True,
        num_devices=NCORES,
    )

    # Per-core inputs (values differ per core; program is SPMD-identical).
    xT = nc.dram_tensor("xT", [D, S], BF16, kind="ExternalInput").ap()
    qT = nc.dram_tensor("qT", [DKS, S], BF16, kind="ExternalInput").ap()
    xres = nc.dram_tensor("xres", [ROWS, D], F32, kind="ExternalInput").ap()
    wk = nc.dram_tensor("wk", [D, DKS], BF16, kind="ExternalInput").ap()
    wv = nc.dram_tensor("wv", [D, DVS], BF16, kind="ExternalInput").ap()
    wf = nc.dram_tensor("wf", [DVS, D], BF16, kind="ExternalInput").ap()
    bkb = nc.dram_tensor("bkb", [1, DKS], BF16, kind="ExternalInput").ap()
    bvb = nc.dram_tensor("bvb", [1, DVS], BF16, kind="ExternalInput").ap()
    maskt = nc.dram_tensor("maskt", [128, 128], BF16, kind="ExternalInput").ap()
    ident = nc.dram_tensor("ident", [128, 128], BF16, kind="ExternalInput").ap()
    onesr = nc.dram_tensor("onesr", [1, 512], BF16, kind="ExternalInput").ap()
    out = nc.dram_tensor("out", [ROWS, D], F32, kind="ExternalOutput").ap()

    with tile.TileContext(nc) as tc:
        _body(nc, tc, xT, qT, xres, wk, wv, wf, bkb, bvb, maskt, ident, onesr, out)

    nc.compile()
    return nc


def _body(nc, tc, xT, qT, xres, wk, wv, wf, bkb, bvb, maskt, ident, onesr, out):
    NST = S // 128     # 16 s tiles of 128
    NQT2 = S // 1024   # 2  q tiles of 1024

    with (
        tc.tile_pool(name="const", bufs=1) as constp,
        tc.tile_pool(name="kv", bufs=1) as kvp,
        tc.tile_pool(name="got", bufs=1) as gotp,
        tc.tile_pool(name="res", bufs=1) as resp,
        tc.tile_pool(name="rfp", bufs=2) as rfp,
        tc.tile_pool(name="small", bufs=8) as smallp,
        tc.tile_pool(name="dram", bufs=1, space="DRAM") as dramp,
    ):
        # ---- constants (ACT queue) ------------------------------------
        ones_sb = constp.tile([1, 512], BF16)
        nc.scalar.dma_start(ones_sb[:], onesr[:])
        mask_sb = constp.tile([128, 128], BF16)
        nc.scalar.dma_start(mask_sb[:], maskt[:])
        bk_sb = constp.tile([1, DKS], BF16)
        nc.scalar.dma_start(bk_sb[:], bkb[:])
        bv_sb = constp.tile([1, DVS], BF16)
        nc.scalar.dma_start(bv_sb[:], bvb[:])

        # Warm up the collectives path (ncfw/channel setup) so the first
        # real ReduceScatter doesn't pay ~25us of first-call overhead.
        warm_in = dramp.tile([4, 16], BF16, tag="warm_in")
        warm_out = dramp.tile([1, 16], BF16, tag="warm_out")
        nc.scalar.dma_start(
            warm_in[:].rearrange("a b -> (a b)")[None, :], ones_sb[0:1, 0:64]
        )
        nc.gpsimd.collective_compute(
            "ReduceScatter",
            mybir.AluOpType.add,
            replica_groups=[[0, 1, 2, 3], [4, 5, 6, 7]],
            ins=[warm_in.opt()],
            outs=[warm_out.opt()],
        )

        # [1024, n] DRAM -> [128, 8*n] SBUF, per-chunk DMAs on the Sync
        # queue (all complete before the first xbar transpose issues)
        def load_chunked(pool, src, n):
            t = pool.tile([128, NM * n], src.dtype)
            for m in range(NM):
                nc.sync.dma_start(
                    t[:, m * n : (m + 1) * n],
                    src[m * 128 : (m + 1) * 128, :],
                )
            return t

        qT_sb = kvp.tile([128, 2 * S], BF16)
        for m in range(2):
            nc.sync.dma_start(
                qT_sb[:, m * S : (m + 1) * S], qT[m * 128 : (m + 1) * 128, :]
            )
        kt_sb = kvp.tile([128, 2 * S], BF16)   # K^T rows dk%128, chunk dk//128
        v_sb = kvp.tile([128, NST * HPC * VSTRIDE], BF16)
        got_sb = gotp.tile([128, NM * S], BF16)  # gelu(o)^T, hc-major x q
        # residual x rows: no deps, load early (ACT queue)
        xrs = []
        for g in range(4):
            xr = resp.tile([128, D], F32, tag=f"xr{g}")
            nc.scalar.dma_start(xr[:], xres[g * 128 : (g + 1) * 128, :])
            xrs.append(xr)

        # ---- projections ---------------------------------------------
        with (
            tc.tile_pool(name="projw", bufs=1) as pwp,
            tc.tile_pool(name="xt", bufs=1) as xtp,
            tc.tile_pool(name="psProj", bufs=4, space="PSUM") as psP,
        ):
            wk_sb = load_chunked(pwp, wk, DKS)
            xT_sb = load_chunked(xtp, xT, S)
            wv_sb = load_chunked(pwp, wv, DVS)

            # K^T[dk, s]: lhsT = Wk chunk [128m, 128dk], rhs = xT chunk [128m, 512s]
            for dkt in range(2):
                for st in range(4):
                    ps = psP.tile([128, 512], F32, tag="proj")
                    nc.tensor.matmul(
                        ps[:],
                        bk_sb[:, dkt * 128 : (dkt + 1) * 128],
                        ones_sb[:, 0:512],
                        start=True,
                        stop=False,
                    )
                    for m in range(NM):
                        nc.tensor.matmul(
                            ps[:],
                            wk_sb[:, m * DKS + dkt * 128 : m * DKS + dkt * 128 + 128],
                            xT_sb[:, m * S + st * 512 : m * S + st * 512 + 512],
                            start=False,
                            stop=(m == NM - 1),
                        )
                    nc.scalar.copy(
                        kt_sb[:, dkt * S + st * 512 : dkt * S + st * 512 + 512], ps[:]
                    )

            # V[s, dv] with a ones column per head (col 256 of each strip)
            nc.vector.memset(
                v_sb[:].rearrange("p (t h c) -> p t h c", t=NST, h=HPC)[:, :, :, DV],
                1.0,
            )
            for st in range(NST):
                for dvh in range(2):  # dv halves of 512 = heads (2*dvh, 2*dvh+1)
                    ps = psP.tile([128, 512], F32, tag="proj")
                    nc.tensor.matmul(
                        ps[:],
                        ones_sb[:, 0:128],
                        bv_sb[:, dvh * 512 : dvh * 512 + 512],
                        start=True,
                        stop=False,
                    )
                    for m in range(NM):
                        nc.tensor.matmul(
                            ps[:],
                            xT_sb[:, m * S + st * 128 : m * S + st * 128 + 128],
                            wv_sb[:, m * DVS + dvh * 512 : m * DVS + dvh * 512 + 512],
                            start=False,
                            stop=(m == NM - 1),
                        )
                    base = st * HPC * VSTRIDE
                    for hh in range(2):
                        h = 2 * dvh + hh
                        nc.scalar.copy(
                            v_sb[:, base + h * VSTRIDE : base + h * VSTRIDE + DV],
                            ps[:, hh * 256 : hh * 256 + 256],
                        )

        # ---- attention (head pairs, row-tiled scores) ----------------
        # scores^T[k, q]: contraction is dk=64, so heads 2p (PE rows 0-63)
        # and 2p+1 (rows 64-127) run concurrently via tile_position row
        # tiling.  AV groups run in default 128x128 mode afterwards;
        # exp without max-subtraction; o tiles transposed by xbar DMA.
        with (
            tc.tile_pool(name="expp", bufs=1) as expp,
            tc.tile_pool(name="otile", bufs=4) as otp,
            tc.tile_pool(name="psSt", bufs=3, space="PSUM") as psS,
            tc.tile_pool(name="psAv", bufs=2, space="PSUM") as psV,
        ):
            NQT2 = S // 1024
            for pair in range(2):
                co = pair * S           # both heads of the pair share chunk co

                def st_tile(j, kt, hl, exps):
                    po = 64 * (hl % 2)
                    t = kt - 8 * j   # >=0 on diagonal k-tiles
                    toff = max(t, 0) * 128
                    q0 = j * 1024 + toff
                    ps = psS.tile([128, 1024], F32, tag="st")
                    lo_w = max(0, 512 - toff)
                    if lo_w:
                        nc.tensor.matmul(
                            ps[:, toff : toff + lo_w],
                            kt_sb[po : po + 64, co + kt * 128 : co + kt * 128 + 128],
                            qT_sb[po : po + 64, co + q0 : co + q0 + lo_w],
                            start=True,
                            stop=True,
                            tile_position=(po, 0),
                        )
                    nc.tensor.matmul(
                        ps[:, max(toff, 512) : 1024],
                        kt_sb[po : po + 64, co + kt * 128 : co + kt * 128 + 128],
                        qT_sb[po : po + 64, co + j * 1024 + max(toff, 512) : co + (j + 1) * 1024],
                        start=True,
                        stop=True,
                        tile_position=(po, 0),
                    )
                    nc.scalar.activation(
                        exps[:, kt * 1024 + toff : (kt + 1) * 1024],
                        ps[:, toff:1024],
                        AF.Exp,
                    )
                    if t >= 0:  # mask the diagonal 128x128 block
                        blk = exps[:, kt * 1024 + toff : kt * 1024 + toff + 128]
                        nc.vector.tensor_mul(blk, blk, mask_sb[:])

                def av_tile(j, sq, hl, exps):
                    i = 8 * j + sq
                    pso = psV.tile([128, VSTRIDE], F32, tag="av")
                    for kt in range(i + 1):
                        vb = kt * HPC * VSTRIDE + hl * VSTRIDE
                        nc.tensor.matmul(
                            pso[:],
                            exps[:, kt * 1024 + sq * 128 : kt * 1024 + sq * 128 + 128],
                            v_sb[:, vb : vb + VSTRIDE],
                            start=(kt == 0),
                            stop=(kt == i),
                        )
                    recip = smallp.tile([128, 1], F32, tag="recip")
                    nc.vector.reciprocal(recip[:], pso[:, DV : DV + 1])
                    ot = otp.tile([128, DV], BF16, tag="ot")
                    nc.vector.tensor_scalar_mul(ot[:], pso[:, 0:DV], recip[:])
                    for half in range(2):
                        hc = 2 * hl + half
                        nc.sync.dma_start_transpose(
                            got_sb[:, hc * S + i * 128 : hc * S + i * 128 + 128],
                            ot[:, half * 128 : half * 128 + 128],
                        )

                for j in range(NQT2):   # 1024-wide q tiles
                    hA, hB = 2 * pair, 2 * pair + 1
                    exps_a = expp.tile([128, 16 * 1024], BF16, tag="expSA")
                    exps_b = expp.tile([128, 16 * 1024], BF16, tag="expSB")
                    # row-tiled score phase: both heads stream concurrently
                    for kt in range(8 * j + 8):
                        st_tile(j, kt, hA, exps_a)
                        st_tile(j, kt, hB, exps_b)
                    # default-mode AV phase
                    for sq in range(8):
                        av_tile(j, sq, hA, exps_a)
                        av_tile(j, sq, hB, exps_b)

        # ---- gelu (exact erf) in place on transposed layout ----------
        for hc in range(NM):
            nc.scalar.activation(
                got_sb[:, hc * S : (hc + 1) * S],
                got_sb[:, hc * S : (hc + 1) * S],
                AF.Gelu,
            )

        # ---- FF partial + chunked ReduceScatter + gpsimd residual ----
        with (
            tc.tile_pool(name="ffw", bufs=1) as ffwp,
            tc.tile_pool(name="ffout", bufs=4) as ffoutp,
            tc.tile_pool(name="psFf", bufs=3, space="PSUM") as psF,
        ):
            wf_sb = load_chunked(ffwp, wf, D)
            for g in range(4):
                partial_d = dramp.tile([512, D], BF16, tag=f"part{g}")
                for cc in range(4):
                    c = 4 * g + cc
                    ps0 = psF.tile([128, 512], F32, tag="ff0")
                    ps1 = psF.tile([128, 512], F32, tag="ff1")
                    for hc in range(NM):
                        lhsT = got_sb[:, hc * S + c * 128 : hc * S + c * 128 + 128]
                        nc.tensor.matmul(
                            ps0[:], lhsT, wf_sb[:, hc * D : hc * D + 512],
                            start=(hc == 0), stop=(hc == NM - 1),
                        )
                        nc.tensor.matmul(
                            ps1[:], lhsT, wf_sb[:, hc * D + 512 : hc * D + 1024],
                            start=(hc == 0), stop=(hc == NM - 1),
                        )
                    fo = ffoutp.tile([128, D], BF16, tag="ffout")
                    nc.vector.tensor_copy(fo[:, 0:512], ps0[:])
                    nc.vector.tensor_copy(fo[:, 512:1024], ps1[:])
                    nc.scalar.dma_start(partial_d[cc * 128 : (cc + 1) * 128, :], fo[:])
                rs_d = dramp.tile([128, D], BF16, tag=f"rs{g}")
                nc.gpsimd.collective_compute(
                    "ReduceScatter",
                    mybir.AluOpType.add,
                    replica_groups=[[0, 1, 2, 3], [4, 5, 6, 7]],
                    ins=[partial_d.opt()],
                    outs=[rs_d.opt()],
                )
                # residual: RS-gated cast-DMA on the GpSimd queue (ordered
                # behind this RS), add on DVE, store on ACT
                rf = rfp.tile([128, D], F32, tag="rf")
                nc.gpsimd.dma_start(rf[:], rs_d[:])
                nc.vector.tensor_add(xrs[g][:], xrs[g][:], rf[:])
                nc.scalar.dma_start(out[g * 128 : (g + 1) * 128, :], xrs[g][:])


def make_in_maps(x, Wk, bk, Wv, bv, Wf, bf):
    """Host-side sharding: returns the per-core input dict list."""
    x = np.asarray(x, np.float32)
    Wk = np.asarray(Wk, np.float32)
    Wv = np.asarray(Wv, np.float32)
    Wf = np.asarray(Wf, np.float32)
    bk = np.asarray(bk, np.float32)
    bv = np.asarray(bv, np.float32)
    bf = np.asarray(bf, np.float32)
    mask = np.tril(np.ones((128, 128), np.float32)).T  # mask[k,q]=1 iff k<=q
    in_maps = []
    for c in range(NCORES):
        b, r = c // GROUP, c % GROUP
        xb = x[b]                                    # [S, D]
        xT = np.ascontiguousarray(xb.T).astype(bf16)
        qTs = xT[DKS * r : DKS * (r + 1)]            # heads 4r..4r+3 rows
        # chunked RS: core (b,r) tile g holds x rows 512g+128r+[0,128)
        xres = np.concatenate(
            [xb[512 * g + 128 * r : 512 * g + 128 * r + 128] for g in range(4)]
        ) + bf[None, :].astype(np.float32)
        in_maps.append({
            "xT": xT,
            "qT": np.ascontiguousarray(qTs),
            "xres": np.ascontiguousarray(xres),
            "wk": np.ascontiguousarray(Wk[:, DKS * r : DKS * (r + 1)]).astype(bf16),
            "wv": np.ascontiguousarray(Wv[:, DVS * r : DVS * (r + 1)]).astype(bf16),
            "wf": np.ascontiguousarray(Wf[DVS * r : DVS * (r + 1), :]).astype(bf16),
            "bkb": bk[None, DKS * r : DKS * (r + 1)].astype(bf16),
            "bvb": bv[None, DVS * r : DVS * (r + 1)].astype(bf16),
            "maskt": mask.astype(bf16),
            "ident": np.eye(128, dtype=np.float32).astype(bf16),
            "onesr": np.ones((1, 512), bf16),
        })
    return in_maps


def assemble(results):
    """[8 x [512,1024]] core outputs -> [2,2048,1024]."""
    out = np.empty((B, S, D), np.float32)
    for c in range(NCORES):
        b, r = c // GROUP, c % GROUP
        for g in range(4):
            out[b, 512 * g + 128 * r : 512 * g + 128 * r + 128, :] = results[c][
                "out"
            ][128 * g : 128 * (g + 1)]
    return out


def kernel(x, Wk, bk, Wv, bv, Wf, bf, _trace=False, _trace_cores=None):
    global _compiled
    if _compiled is None:
        _compiled = build_program()
    nc = _compiled
    in_maps = make_in_maps(x, Wk, bk, Wv, bv, Wf, bf)
    res = bass_utils.run_bass_kernel_spmd(
        nc,
        in_maps,
        core_ids=list(range(NCORES)),
        trace=_trace,
        trace_cores=_trace_cores,
    )
    out = assemble(res.results)
    kernel.last_result = res
    return out
